# revision 1
# baseline (speedup 1.0000x reference)
"""Trainium2 Bass kernel for nn_Conv4Pim_group_arr_v3 (PIM-style grouped quantized conv).

Computation (see reference):
  - x [16,256,56,56] f32, weight [256,256,3,3], per-group (G=4, 64 ic each) LSQ
    quantization: weights to integer levels {0..3} (pos/neg split), partial-sum conv
    outputs rounded to int levels in [-128,127] and rescaled, accumulated over groups.

Strategy: data-parallel over batch (2 images per core, 8 cores, no collectives).
Per core, per (img, group, och-tile-of-512, sptile-of-8-rows):
  - 5 fp16 matmuls (K=128: two conv taps x 64 ic stacked; x stored as [A | A+1] and
    [A | A+58] shifted copies so taps pair up) accumulate the 3x3 conv into one
    PSUM tile [128 och, 464=8*58 padded-row columns].
  - ACT: Copy(psum * (w_scale/ps_scale)) with int8 output = round-half-even +
    saturate to [-128,127] in one op (verified on HW) == the LSQ psum quantizer.
  - DVE scalar_tensor_tensor: acc_fp16 += q_int8 * (+-ps_scale).
Output fp16 -> host f32.
"""

import numpy as np

import concourse.mybir as mybir
import concourse.tile as tile
from concourse import bacc
from concourse.bass_utils import run_bass_kernel_spmd

F32 = mybir.dt.float32
F16 = mybir.dt.float16
I8 = mybir.dt.int8

B, IC, H, W = 16, 256, 56, 56
OC = 256
G = 4
CG = 64  # ic per group
K = 3
QP_W = 3  # 2**2 - 1
N_CORES = 8
BPC = B // N_CORES  # images per core

PW = W + 2  # 58 padded width
PH = H + 2
FLAT = PW * PH  # 3364
FLATP = FLAT + 4  # padded to 3368 for tap-read overhang
SP = 7  # spatial tiles of 8 output rows
ROWS = 8
NCOL = ROWS * PW  # 464 columns per psum tile
OC4 = 4  # och tiles of 128 over 512 (pos|neg x 256)

_nc_cache = {}


def _build_nc():
    nc = bacc.Bacc(
        "TRN2",
        target_bir_lowering=False,
        debug=False,
        enable_asserts=True,
        num_devices=N_CORES,
    )

    xt1_d = nc.dram_tensor("xt1", [BPC, G, 128, FLATP], F16, kind="ExternalInput").ap()
    xt2_d = nc.dram_tensor("xt2", [BPC, G, 128, FLATP], F16, kind="ExternalInput").ap()
    wts_d = nc.dram_tensor("wts", [128, G * OC4 * 5 * 128], F16, kind="ExternalInput").ap()
    scl_d = nc.dram_tensor("scl", [128, 2 * G * OC4], F32, kind="ExternalInput").ap()
    # padded output: [img, oct, sp, och, 464 cols]; host strips the pad columns
    out_d = nc.dram_tensor("out", [BPC, 2, SP, 128, NCOL], F16, kind="ExternalOutput").ap()

    with tile.TileContext(nc) as tc:
        with (
            tc.tile_pool(name="xp", bufs=1) as xp,
            tc.tile_pool(name="wp", bufs=1) as wp,
            tc.tile_pool(name="accp", bufs=2) as accp,
            tc.tile_pool(name="qp", bufs=8) as qp,
            tc.tile_pool(name="psum", bufs=8, space="PSUM") as pp,
        ):
            wts = wp.tile([128, G * OC4 * 5 * 128], F16, tag="wts")
            scl = wp.tile([128, 2 * G * OC4], F32, tag="scl")
            # Startup-critical DMA schedule over two queues (sync = HWDGE, gpsimd =
            # SWDGE). The first (img0, g0) block runs sp-outer (see compute loop), so
            # only the first ~1100 cols of T1/T2[0,0] + the (g0,oc4=0) weight slice
            # gate the first matmul. Chunks ordered to stay ahead of consumption.
            W1 = 5 * 128  # one (g, oc4) weight slice
            WG = OC4 * W1  # one group
            C1, C2 = 1100, 2500  # x-tile column chunk boundaries

            xt = {}
            t1_first = xp.tile([128, FLATP], F16, tag="t1_0_0")
            t2_first = xp.tile([128, FLATP], F16, tag="t2_0_0")
            xt[0, 0] = (t1_first, t2_first)

            nc.sync.dma_start(scl[:], scl_d[:])
            nc.sync.dma_start(wts[:, :W1], wts_d[:, :W1])
            nc.sync.dma_start(t1_first[:, :C1], xt1_d[0, 0, :, :C1])
            for i in range(1, OC4):
                nc.sync.dma_start(wts[:, i * W1 : (i + 1) * W1], wts_d[:, i * W1 : (i + 1) * W1])
            nc.sync.dma_start(t1_first[:, C1:C2], xt1_d[0, 0, :, C1:C2])
            nc.sync.dma_start(t1_first[:, C2:], xt1_d[0, 0, :, C2:])

            nc.gpsimd.dma_start(t2_first[:, :C1], xt2_d[0, 0, :, :C1])
            nc.gpsimd.dma_start(t2_first[:, C1:C2], xt2_d[0, 0, :, C1:C2])
            nc.gpsimd.dma_start(t2_first[:, C2:], xt2_d[0, 0, :, C2:])
            nc.gpsimd.dma_start(wts[:, WG : 2 * WG], wts_d[:, WG : 2 * WG])

            for img in range(BPC):
                for g in range(G):
                    if (img, g) in xt:
                        continue
                    t1 = xp.tile([128, FLATP], F16, tag=f"t1_{img}_{g}")
                    t2 = xp.tile([128, FLATP], F16, tag=f"t2_{img}_{g}")
                    nc.sync.dma_start(t1[:], xt1_d[img, g])
                    nc.gpsimd.dma_start(t2[:], xt2_d[img, g])
                    xt[img, g] = (t1, t2)
                    if (img, g) == (0, 1):
                        # remaining weights after the (0,1) x tiles
                        nc.gpsimd.dma_start(wts[:, 2 * WG :], wts_d[:, 2 * WG :])

            def wslice(g, oc4, s):
                i = ((g * OC4) + oc4) * 5 + s
                return wts[:, i * 128 : (i + 1) * 128]

            for img in range(BPC):
                acc = {}
                for oct in range(2):
                    for sp in range(SP):
                        a_t = accp.tile([128, NCOL], F16, tag=f"acc{oct}_{sp}")
                        acc[oct, sp] = a_t

                for g in range(G):
                    t1, t2 = xt[img, g]
                    if img == 0 and g == 0:
                        # sp-outer so the first matmuls need only the first x chunk
                        combos = [(oc4, sp) for sp in range(SP) for oc4 in range(OC4)]
                    else:
                        combos = [(oc4, sp) for oc4 in range(OC4) for sp in range(SP)]
                    for oc4, sp in combos:
                        iscl = g * OC4 + oc4
                        ratio_ap = scl[:, iscl : iscl + 1]
                        c_ap = scl[:, G * OC4 + iscl : G * OC4 + iscl + 1]
                        if True:
                            r0 = sp * ROWS
                            ps = pp.tile([128, NCOL], F32, tag="ps")
                            for s in range(3):
                                nc.tensor.matmul(
                                    ps[:],
                                    wslice(g, oc4, s),
                                    t1[:, (r0 + s) * PW : (r0 + s) * PW + NCOL],
                                    start=(s == 0),
                                    stop=False,
                                )
                            nc.tensor.matmul(
                                ps[:],
                                wslice(g, oc4, 3),
                                t2[:, r0 * PW + 2 : r0 * PW + 2 + NCOL],
                                start=False,
                                stop=False,
                            )
                            nc.tensor.matmul(
                                ps[:],
                                wslice(g, oc4, 4),
                                t1[:, (r0 + 2) * PW + 2 : (r0 + 2) * PW + 2 + NCOL],
                                start=False,
                                stop=True,
                            )
                            q8 = qp.tile([128, NCOL], I8, tag="q8")
                            nc.scalar.activation(
                                q8[:],
                                ps[:],
                                mybir.ActivationFunctionType.Copy,
                                bias=0.0,
                                scale=ratio_ap,
                            )
                            a = acc[oc4 % 2, sp]
                            if g == 0 and oc4 < 2:
                                nc.vector.tensor_scalar(
                                    a[:], q8[:], c_ap, None, mybir.AluOpType.mult
                                )
                            else:
                                nc.vector.scalar_tensor_tensor(
                                    a[:],
                                    q8[:],
                                    c_ap,
                                    a[:],
                                    mybir.AluOpType.mult,
                                    mybir.AluOpType.add,
                                )

                for oct in range(2):
                    for sp in range(SP):
                        a = acc[oct, sp]
                        eng = nc.sync if (oct * SP + sp) % 2 == 0 else nc.gpsimd
                        eng.dma_start(out_d[img, oct, sp], a[:])

    nc.compile()
    return nc


def _prepare(x, weight, w_scale, ps_scale_p, ps_scale_n):
    x = np.asarray(x, np.float32)
    weight = np.asarray(weight, np.float32)
    w_scale = np.asarray(w_scale, np.float32)
    ps_scale_p = np.asarray(ps_scale_p, np.float32)
    ps_scale_n = np.asarray(ps_scale_n, np.float32)

    # --- weight levels (exact f32 math matching the reference LSQ) ---
    wg = weight.reshape(OC, G, CG, K, K).transpose(1, 0, 2, 3, 4)  # [G,O,cg,k,k]
    s_w = w_scale.reshape(G, 1, 1, 1, 1)
    lvl_p = np.round(np.clip(np.maximum(wg, 0) / s_w, 0.0, float(QP_W))).astype(np.float32)
    lvl_n = np.round(np.clip(np.maximum(-wg, 0) / s_w, 0.0, float(QP_W))).astype(np.float32)
    LV = np.concatenate([lvl_p, lvl_n], axis=1)  # [G, 512, cg, 3, 3]

    # lhsT tiles [K=128, M=128] per (g, oc4, slot)
    wts = np.zeros((G, OC4, 5, 128, 128), np.float16)
    for g in range(G):
        for oc4 in range(OC4):
            t = LV[g, oc4 * 128 : (oc4 + 1) * 128]  # [128 och, cg, 3, 3]
            for s in range(3):  # taps (s,0)+(s,1)
                wts[g, oc4, s, :CG] = t[:, :, s, 0].T
                wts[g, oc4, s, CG:] = t[:, :, s, 1].T
            wts[g, oc4, 3, :CG] = t[:, :, 0, 2].T  # taps (0,2)+(1,2) via T2
            wts[g, oc4, 3, CG:] = t[:, :, 1, 2].T
            wts[g, oc4, 4, :CG] = t[:, :, 2, 2].T  # tap (2,2), upper half zero
    # -> [128 K, G*OC4*5*128]
    wts_flat = np.ascontiguousarray(wts.transpose(3, 0, 1, 2, 4).reshape(128, G * OC4 * 5 * 128))

    # --- scales: ratio = s_w/s_ps ; c = +-s_ps ---
    scl = np.zeros((128, 2 * G * OC4), np.float32)
    for g in range(G):
        for oc4 in range(OC4):
            s_ps = ps_scale_p[g] if oc4 < 2 else ps_scale_n[g]
            sign = 1.0 if oc4 < 2 else -1.0
            scl[:, g * OC4 + oc4] = np.float32(w_scale[g]) / np.float32(s_ps)
            scl[:, G * OC4 + g * OC4 + oc4] = np.float32(sign) * np.float32(s_ps)

    # --- padded, shifted x in fp16 ---
    xp = np.zeros((B, IC, PH, PW), np.float16)
    xp[:, :, 1 : H + 1, 1 : W + 1] = x.astype(np.float16)
    Af = np.zeros((B, G, CG, FLATP), np.float16)
    Af[..., :FLAT] = xp.reshape(B, G, CG, FLAT)
    T1 = np.zeros((B, G, 128, FLATP), np.float16)
    T1[:, :, :CG] = Af
    T1[:, :, CG:, : FLATP - 1] = Af[..., 1:]
    T2 = np.zeros((B, G, 128, FLATP), np.float16)
    T2[:, :, :CG] = Af
    T2[:, :, CG:, : FLATP - PW] = Af[..., PW:]

    return T1, T2, wts_flat, scl


def kernel(x, weight, w_scale, ps_scale_p, ps_scale_n, _trace=False, _tmpdir=None):
    T1, T2, wts_flat, scl = _prepare(x, weight, w_scale, ps_scale_p, ps_scale_n)

    if "nc" not in _nc_cache:
        _nc_cache["nc"] = _build_nc()
    nc = _nc_cache["nc"]

    in_maps = []
    for c in range(N_CORES):
        sl = slice(c * BPC, (c + 1) * BPC)
        in_maps.append(
            {
                "xt1": np.ascontiguousarray(T1[sl]),
                "xt2": np.ascontiguousarray(T2[sl]),
                "wts": wts_flat,
                "scl": scl,
            }
        )

    kwargs = {}
    if _trace:
        kwargs.update(trace=True, tmpdir=_tmpdir, trace_cores=[0])
    res = run_bass_kernel_spmd(nc, in_maps, core_ids=list(range(N_CORES)), **kwargs)

    out = np.concatenate([r["out"] for r in res.results], axis=0)  # [16, 2, 7, 128, 464] fp16
    v = out.reshape(B, 2, SP, 128, ROWS, PW)[..., :W]  # strip pad cols
    final = np.ascontiguousarray(v.transpose(0, 1, 3, 2, 4, 5)).reshape(B, OC, H, W).astype(np.float32)
    if _trace:
        kernel._last_results = res
    return final



# revision 3
# speedup vs baseline: 1.0131x; 1.0131x over previous
"""Trainium2 Bass kernel for nn_Conv4Pim_group_arr_v3 (PIM-style grouped quantized conv).

Computation (see reference):
  - x [16,256,56,56] f32, weight [256,256,3,3], per-group (G=4, 64 ic each) LSQ
    quantization: weights to integer levels {0..3} (pos/neg split), partial-sum conv
    outputs rounded to int levels in [-128,127] and rescaled, accumulated over groups.

Strategy: data-parallel over batch (2 images per core, 8 cores, no collectives).
fp8 DoubleRow matmuls: x is split exactly into two e4m3 lanes (x ~ a/4 + b/128 with
a,b integers in [-16,16], both lanes exact in fp8), weights are integer levels {0..3}
(exact in fp8). Each DoubleRow pass contracts both lanes (2 K-tiles of 128) at double
rate, so the 5-pass-per-tile structure of the fp16 kernel keeps its shape but the
tensor engine runs at the fp8 rate.

Per core, per (img, group, och-tile-of-512, sptile-of-8-rows):
  - 5 fp8 DoubleRow matmuls (K=2x128: two conv taps x 64 ic stacked per K-tile;
    x stored as [A | A+1] and [A | A+58] shifted copies so taps pair up) accumulate
    the 3x3 conv into one PSUM tile [128 och, 464=8*58 padded-row columns].
  - ACT: Copy(psum * (w_scale/ps_scale)) with int8 output = round-half-even +
    saturate to [-128,127] in one op == the LSQ psum quantizer.
  - DVE scalar_tensor_tensor: acc_fp16 += q_int8 * (+-ps_scale).
Output fp16 -> host f32.
"""

import numpy as np
import ml_dtypes

import concourse.mybir as mybir
import concourse.tile as tile
from concourse import bacc
from concourse.bass_utils import run_bass_kernel_spmd

F32 = mybir.dt.float32
F16 = mybir.dt.float16
F8 = mybir.dt.float8e4
I8 = mybir.dt.int8
NP_F8 = ml_dtypes.float8_e4m3

B, IC, H, W = 16, 256, 56, 56
OC = 256
G = 4
CG = 64  # ic per group
K = 3
QP_W = 3  # 2**2 - 1
N_CORES = 8
BPC = B // N_CORES  # images per core

PW = W + 2  # 58 padded width
PH = H + 2
FLAT = PW * PH  # 3364
FLATP = FLAT + 4  # padded to 3368 for tap-read overhang
SP = 7  # spatial tiles of 8 output rows
ROWS = 8
NCOL = ROWS * PW  # 464 columns per psum tile
OC4 = 4  # och tiles of 128 over 512 (pos|neg x 256)
NSL = G * OC4 * 5  # weight slices, each [128 K, 2 lanes, 128 M]

_nc_cache = {}


def _build_nc():
    nc = bacc.Bacc(
        "TRN2",
        target_bir_lowering=False,
        debug=False,
        enable_asserts=True,
        num_devices=N_CORES,
    )

    xt1_d = nc.dram_tensor("xt1", [BPC, G, 128, 2, FLATP], F8, kind="ExternalInput").ap()
    xt2_d = nc.dram_tensor("xt2", [BPC, G, 128, 2, FLATP], F8, kind="ExternalInput").ap()
    wts_d = nc.dram_tensor("wts", [128, NSL, 2, 128], F8, kind="ExternalInput").ap()
    scl_d = nc.dram_tensor("scl", [128, 2 * G * OC4], F32, kind="ExternalInput").ap()
    # padded output: [img, oct, sp, och, 464 cols]; host strips the pad columns
    out_d = nc.dram_tensor("out", [BPC, 2, SP, 128, NCOL], F16, kind="ExternalOutput").ap()

    DR = mybir.MatmulPerfMode.DoubleRow

    with tile.TileContext(nc) as tc:
        with (
            tc.tile_pool(name="xp", bufs=1) as xp,
            tc.tile_pool(name="wp", bufs=1) as wp,
            tc.tile_pool(name="accp", bufs=2) as accp,
            tc.tile_pool(name="qp", bufs=8) as qp,
            tc.tile_pool(name="psum", bufs=8, space="PSUM") as pp,
        ):
            wts = wp.tile([128, NSL, 2, 128], F8, tag="wts")
            scl = wp.tile([128, 2 * G * OC4], F32, tag="scl")
            # Startup-critical DMA schedule over two queues (sync = HWDGE, gpsimd =
            # SWDGE). The first (img0, g0) block runs sp-outer (see compute loop), so
            # only the first ~1100 cols of T1/T2[0,0] + the (g0,oc4=0) weight slice
            # gate the first matmul. Chunks ordered to stay ahead of consumption.
            W1 = 5  # one (g, oc4) weight slice group (5 slots)
            WG = OC4 * W1  # one group
            C1, C2 = 1100, 2500  # x-tile column chunk boundaries

            xt = {}
            t1_first = xp.tile([128, 2, FLATP], F8, tag="t1_0_0")
            t2_first = xp.tile([128, 2, FLATP], F8, tag="t2_0_0")
            xt[0, 0] = (t1_first, t2_first)

            nc.sync.dma_start(scl[:], scl_d[:])
            nc.sync.dma_start(wts[:, :W1], wts_d[:, :W1])
            nc.sync.dma_start(t1_first[:, :, :C1], xt1_d[0, 0, :, :, :C1])
            for i in range(1, OC4):
                nc.sync.dma_start(wts[:, i * W1 : (i + 1) * W1], wts_d[:, i * W1 : (i + 1) * W1])
            nc.sync.dma_start(t1_first[:, :, C1:C2], xt1_d[0, 0, :, :, C1:C2])
            nc.sync.dma_start(t1_first[:, :, C2:], xt1_d[0, 0, :, :, C2:])

            nc.gpsimd.dma_start(t2_first[:, :, :C1], xt2_d[0, 0, :, :, :C1])
            nc.gpsimd.dma_start(t2_first[:, :, C1:C2], xt2_d[0, 0, :, :, C1:C2])
            nc.gpsimd.dma_start(t2_first[:, :, C2:], xt2_d[0, 0, :, :, C2:])
            nc.gpsimd.dma_start(wts[:, WG : 2 * WG], wts_d[:, WG : 2 * WG])

            for img in range(BPC):
                for g in range(G):
                    if (img, g) in xt:
                        continue
                    t1 = xp.tile([128, 2, FLATP], F8, tag=f"t1_{img}_{g}")
                    t2 = xp.tile([128, 2, FLATP], F8, tag=f"t2_{img}_{g}")
                    nc.sync.dma_start(t1[:], xt1_d[img, g])
                    nc.gpsimd.dma_start(t2[:], xt2_d[img, g])
                    xt[img, g] = (t1, t2)
                    if (img, g) == (0, 1):
                        # remaining weights after the (0,1) x tiles
                        nc.gpsimd.dma_start(wts[:, 2 * WG :], wts_d[:, 2 * WG :])

            def wslice(g, oc4, s):
                i = ((g * OC4) + oc4) * 5 + s
                return wts[:, i]

            for img in range(BPC):
                acc = {}
                for oct in range(2):
                    for sp in range(SP):
                        a_t = accp.tile([128, NCOL], F16, tag=f"acc{oct}_{sp}")
                        acc[oct, sp] = a_t

                for g in range(G):
                    t1, t2 = xt[img, g]
                    if img == 0 and g == 0:
                        # sp-outer so the first matmuls need only the first x chunk
                        combos = [(oc4, sp) for sp in range(SP) for oc4 in range(OC4)]
                    else:
                        combos = [(oc4, sp) for oc4 in range(OC4) for sp in range(SP)]
                    for oc4, sp in combos:
                        iscl = g * OC4 + oc4
                        ratio_ap = scl[:, iscl : iscl + 1]
                        c_ap = scl[:, G * OC4 + iscl : G * OC4 + iscl + 1]
                        if True:
                            r0 = sp * ROWS
                            ps = pp.tile([128, NCOL], F32, tag="ps")
                            for s in range(3):
                                nc.tensor.matmul(
                                    ps[:],
                                    wslice(g, oc4, s),
                                    t1[:, :, (r0 + s) * PW : (r0 + s) * PW + NCOL],
                                    start=(s == 0),
                                    stop=False,
                                    perf_mode=DR,
                                )
                            nc.tensor.matmul(
                                ps[:],
                                wslice(g, oc4, 3),
                                t2[:, :, r0 * PW + 2 : r0 * PW + 2 + NCOL],
                                start=False,
                                stop=False,
                                perf_mode=DR,
                            )
                            nc.tensor.matmul(
                                ps[:],
                                wslice(g, oc4, 4),
                                t1[:, :, (r0 + 2) * PW + 2 : (r0 + 2) * PW + 2 + NCOL],
                                start=False,
                                stop=True,
                                perf_mode=DR,
                            )
                            q8 = qp.tile([128, NCOL], I8, tag="q8")
                            nc.scalar.activation(
                                q8[:],
                                ps[:],
                                mybir.ActivationFunctionType.Copy,
                                bias=0.0,
                                scale=ratio_ap,
                            )
                            a = acc[oc4 % 2, sp]
                            if g == 0 and oc4 < 2:
                                nc.vector.tensor_scalar(
                                    a[:], q8[:], c_ap, None, mybir.AluOpType.mult
                                )
                            else:
                                nc.vector.scalar_tensor_tensor(
                                    a[:],
                                    q8[:],
                                    c_ap,
                                    a[:],
                                    mybir.AluOpType.mult,
                                    mybir.AluOpType.add,
                                )

                for oct in range(2):
                    for sp in range(SP):
                        a = acc[oct, sp]
                        eng = nc.sync if (oct * SP + sp) % 2 == 0 else nc.gpsimd
                        eng.dma_start(out_d[img, oct, sp], a[:])

    nc.compile()
    return nc


def _prepare(x, weight, w_scale, ps_scale_p, ps_scale_n):
    x = np.asarray(x, np.float32)
    weight = np.asarray(weight, np.float32)
    w_scale = np.asarray(w_scale, np.float32)
    ps_scale_p = np.asarray(ps_scale_p, np.float32)
    ps_scale_n = np.asarray(ps_scale_n, np.float32)

    # --- weight levels (exact f32 math matching the reference LSQ) ---
    wg = weight.reshape(OC, G, CG, K, K).transpose(1, 0, 2, 3, 4)  # [G,O,cg,k,k]
    s_w = w_scale.reshape(G, 1, 1, 1, 1)
    lvl_p = np.round(np.clip(np.maximum(wg, 0) / s_w, 0.0, float(QP_W))).astype(np.float32)
    lvl_n = np.round(np.clip(np.maximum(-wg, 0) / s_w, 0.0, float(QP_W))).astype(np.float32)
    LV = np.concatenate([lvl_p, lvl_n], axis=1)  # [G, 512, cg, 3, 3]

    # lhsT tiles [K=128, M=128] per (g, oc4, slot); both DoubleRow lanes get the
    # same integer-level weights (lane 0 contracts x_hi, lane 1 x_lo).
    wts = np.zeros((G, OC4, 5, 128, 128), np.float32)
    for g in range(G):
        for oc4 in range(OC4):
            t = LV[g, oc4 * 128 : (oc4 + 1) * 128]  # [128 och, cg, 3, 3]
            for s in range(3):  # taps (s,0)+(s,1)
                wts[g, oc4, s, :CG] = t[:, :, s, 0].T
                wts[g, oc4, s, CG:] = t[:, :, s, 1].T
            wts[g, oc4, 3, :CG] = t[:, :, 0, 2].T  # taps (0,2)+(1,2) via T2
            wts[g, oc4, 3, CG:] = t[:, :, 1, 2].T
            wts[g, oc4, 4, :CG] = t[:, :, 2, 2].T  # tap (2,2), upper half zero
    # -> [128 K, NSL, 2 lanes, 128 M]
    wflat = wts.transpose(3, 0, 1, 2, 4).reshape(128, NSL, 1, 128)
    wts_flat = np.ascontiguousarray(
        np.broadcast_to(wflat, (128, NSL, 2, 128))
    ).astype(NP_F8)

    # --- scales: ratio = s_w/s_ps ; c = +-s_ps ---
    scl = np.zeros((128, 2 * G * OC4), np.float32)
    for g in range(G):
        for oc4 in range(OC4):
            s_ps = ps_scale_p[g] if oc4 < 2 else ps_scale_n[g]
            sign = 1.0 if oc4 < 2 else -1.0
            scl[:, g * OC4 + oc4] = np.float32(w_scale[g]) / np.float32(s_ps)
            scl[:, G * OC4 + g * OC4 + oc4] = np.float32(sign) * np.float32(s_ps)

    # --- exact two-lane fp8 split of x: x ~ a/4 + b/128, lanes e4m3-exact ---
    # hi lane: integers/4 in [-4,4]; tail |x|>4.125 uses even ints (exact in
    # e4m3 up to 32) at step 1/2, with the lo lane also on an even grid there.
    a4 = np.round(x * 4)
    big = np.abs(a4) > 16
    a4 = np.where(big, 2 * np.round(x * 2), a4)
    a = a4.astype(np.float32) / 4
    r = x - a
    b128 = np.clip(np.round(r * 128), -16, 16)
    b128 = np.where(big, 2 * np.round(r * 64), b128)
    b = b128.astype(np.float32) / 128
    # padded, shifted lanes: [B, G, 128 part, 2 lane, FLATP]
    T1 = np.zeros((B, G, 128, 2, FLATP), NP_F8)
    T2 = np.zeros((B, G, 128, 2, FLATP), NP_F8)
    for lane, xl in ((0, a), (1, b)):
        xp8 = np.zeros((B, IC, PH, PW), NP_F8)
        xp8[:, :, 1 : H + 1, 1 : W + 1] = xl.astype(NP_F8)
        Af = np.zeros((B, G, CG, FLATP), NP_F8)
        Af[..., :FLAT] = xp8.reshape(B, G, CG, FLAT)
        T1[:, :, :CG, lane] = Af
        T1[:, :, CG:, lane, : FLATP - 1] = Af[..., 1:]
        T2[:, :, :CG, lane] = Af
        T2[:, :, CG:, lane, : FLATP - PW] = Af[..., PW:]

    return T1, T2, wts_flat, scl


def kernel(x, weight, w_scale, ps_scale_p, ps_scale_n, _trace=False, _tmpdir=None):
    T1, T2, wts_flat, scl = _prepare(x, weight, w_scale, ps_scale_p, ps_scale_n)

    if "nc" not in _nc_cache:
        _nc_cache["nc"] = _build_nc()
    nc = _nc_cache["nc"]

    in_maps = []
    for c in range(N_CORES):
        sl = slice(c * BPC, (c + 1) * BPC)
        in_maps.append(
            {
                "xt1": np.ascontiguousarray(T1[sl]),
                "xt2": np.ascontiguousarray(T2[sl]),
                "wts": wts_flat,
                "scl": scl,
            }
        )

    kwargs = {}
    if _trace:
        kwargs.update(trace=True, tmpdir=_tmpdir, trace_cores=[0])
    res = run_bass_kernel_spmd(nc, in_maps, core_ids=list(range(N_CORES)), **kwargs)

    out = np.concatenate([r["out"] for r in res.results], axis=0)  # [16, 2, 7, 128, 464] fp16
    v = out.reshape(B, 2, SP, 128, ROWS, PW)[..., :W]  # strip pad cols
    final = np.ascontiguousarray(v.transpose(0, 1, 3, 2, 4, 5)).reshape(B, OC, H, W).astype(np.float32)
    if _trace:
        kernel._last_results = res
    return final


# revision 4
# speedup vs baseline: 1.0524x; 1.0388x over previous
"""Trainium2 Bass kernel for nn_Conv4Pim_group_arr_v3 (PIM-style grouped quantized conv).

Computation (see reference):
  - x [16,256,56,56] f32, weight [256,256,3,3], per-group (G=4, 64 ic each) LSQ
    quantization: weights to integer levels {0..3} (pos/neg split), partial-sum conv
    outputs rounded to int levels in [-128,127] and rescaled, accumulated over groups.

Strategy: data-parallel over batch (2 images per core, 8 cores, no collectives).
fp8 DoubleRow matmuls: x is split exactly into two e4m3 lanes (x ~ a/4 + b/128 with
a,b integers, both lanes exact in fp8; |x|>4.125 tail uses an even grid, still exact),
weights are integer levels {0..3} (exact in fp8). Each DoubleRow pass contracts both
lanes (2 K-tiles of 128) per column, so full x precision costs no extra passes.

x is stored as a padded [58 rows, 58 cols] grid per (img, group), two shifted copies
(T1 = [A | A+1col] and T2 = [A | A+1row] stacked in the K partition dim) so conv taps
pair up. All matmul reads are rectangular [8 rows, 56 cols] slices -> 448 columns per
pass (no pad-column waste).

Per core, per (img, group, och-tile-of-512, sptile-of-8-rows):
  - 5 fp8 DoubleRow matmuls accumulate the 3x3 conv into one PSUM tile [128, 448].
  - ACT: Copy(psum * (w_scale/ps_scale)) with int8 output = round-half-even +
    saturate to [-128,127] in one op == the LSQ psum quantizer.
  - DVE scalar_tensor_tensor: acc_fp16 += q_int8 * (+-ps_scale).
Output fp16 -> host f32.
"""

import numpy as np
import ml_dtypes

import concourse.mybir as mybir
import concourse.tile as tile
from concourse import bacc
from concourse.bass_utils import run_bass_kernel_spmd

F32 = mybir.dt.float32
F16 = mybir.dt.float16
F8 = mybir.dt.float8e4
I8 = mybir.dt.int8
NP_F8 = ml_dtypes.float8_e4m3

B, IC, H, W = 16, 256, 56, 56
OC = 256
G = 4
CG = 64  # ic per group
K = 3
QP_W = 3  # 2**2 - 1
N_CORES = 8
BPC = B // N_CORES  # images per core

PW = W + 2  # 58 padded width
PH = H + 2  # 58 padded height
FLAT = PW * PH  # 3364
SP = 7  # spatial tiles of 8 output rows
ROWS = 8
NCOL = ROWS * W  # 448 columns per psum tile
OC4 = 4  # och tiles of 128 over 512 (pos|neg x 256)
NSL = G * OC4 * 5  # weight slices, each [128 K, 2 lanes, 128 M]

_nc_cache = {}


def _build_nc():
    nc = bacc.Bacc(
        "TRN2",
        target_bir_lowering=False,
        debug=False,
        enable_asserts=True,
        num_devices=N_CORES,
    )

    xt1_d = nc.dram_tensor("xt1", [BPC, G, 128, 2, PH, PW], F8, kind="ExternalInput").ap()
    xt2_d = nc.dram_tensor("xt2", [BPC, G, 128, 2, PH, PW], F8, kind="ExternalInput").ap()
    wts_d = nc.dram_tensor("wts", [128, NSL, 2, 128], F8, kind="ExternalInput").ap()
    scl_d = nc.dram_tensor("scl", [128, 2 * G * OC4], F32, kind="ExternalInput").ap()
    out_d = nc.dram_tensor("out", [BPC, 2, SP, 128, NCOL], F16, kind="ExternalOutput").ap()

    DR = mybir.MatmulPerfMode.DoubleRow

    with tile.TileContext(nc) as tc:
        with (
            tc.tile_pool(name="xp", bufs=1) as xp,
            tc.tile_pool(name="wp", bufs=1) as wp,
            tc.tile_pool(name="accp", bufs=2) as accp,
            tc.tile_pool(name="qp", bufs=8) as qp,
            tc.tile_pool(name="psum", bufs=8, space="PSUM") as pp,
        ):
            wts = wp.tile([128, NSL, 2, 128], F8, tag="wts")
            scl = wp.tile([128, 2 * G * OC4], F32, tag="scl")
            # Startup-critical DMA schedule over two queues (sync = HWDGE, gpsimd =
            # SWDGE). The (img0, g0) block runs oc4-outer: each oc4 sweep (7 sp tiles,
            # ~12us) needs one 5-slot weight slice and consumes x rows progressively,
            # so row-chunked first tiles + the (g0,oc4=0) weight slots gate startup.
            W1 = 5  # slots per (g, oc4) weight slice
            WG = OC4 * W1  # slots per group
            R1, R2, R3 = 10, 26, 42  # x-tile row chunk boundaries

            xt = {}
            t1_first = xp.tile([128, 2, PH, PW], F8, tag="t1_0_0")
            t2_first = xp.tile([128, 2, PH, PW], F8, tag="t2_0_0")
            xt[0, 0] = (t1_first, t2_first)

            nc.sync.dma_start(wts[:, 0:1], wts_d[:, 0:1])
            nc.sync.dma_start(t1_first[:, :, :R1], xt1_d[0, 0, :, :, :R1])
            nc.sync.dma_start(wts[:, 1:W1], wts_d[:, 1:W1])
            nc.sync.dma_start(scl[:], scl_d[:])
            nc.sync.dma_start(t1_first[:, :, R1:R2], xt1_d[0, 0, :, :, R1:R2])
            nc.sync.dma_start(t1_first[:, :, R2:R3], xt1_d[0, 0, :, :, R2:R3])
            nc.sync.dma_start(t1_first[:, :, R3:], xt1_d[0, 0, :, :, R3:])
            for i in range(2, OC4):
                nc.sync.dma_start(wts[:, i * W1 : (i + 1) * W1], wts_d[:, i * W1 : (i + 1) * W1])

            nc.gpsimd.dma_start(t2_first[:, :, :R1], xt2_d[0, 0, :, :, :R1])
            nc.gpsimd.dma_start(t2_first[:, :, R1:R2], xt2_d[0, 0, :, :, R1:R2])
            nc.gpsimd.dma_start(wts[:, W1 : 2 * W1], wts_d[:, W1 : 2 * W1])
            nc.gpsimd.dma_start(t2_first[:, :, R2:R3], xt2_d[0, 0, :, :, R2:R3])
            nc.gpsimd.dma_start(t2_first[:, :, R3:], xt2_d[0, 0, :, :, R3:])
            nc.gpsimd.dma_start(wts[:, WG : 2 * WG], wts_d[:, WG : 2 * WG])

            for img in range(BPC):
                for g in range(G):
                    if (img, g) in xt:
                        continue
                    t1 = xp.tile([128, 2, PH, PW], F8, tag=f"t1_{img}_{g}")
                    t2 = xp.tile([128, 2, PH, PW], F8, tag=f"t2_{img}_{g}")
                    nc.sync.dma_start(t1[:], xt1_d[img, g])
                    nc.gpsimd.dma_start(t2[:], xt2_d[img, g])
                    xt[img, g] = (t1, t2)
                    if (img, g) == (0, 1):
                        # remaining weights after the (0,1) x tiles
                        nc.gpsimd.dma_start(wts[:, 2 * WG :], wts_d[:, 2 * WG :])

            def wslice(g, oc4, s):
                i = ((g * OC4) + oc4) * 5 + s
                return wts[:, i]

            for img in range(BPC):
                acc = {}
                for oct in range(2):
                    for sp in range(SP):
                        a_t = accp.tile([128, NCOL], F16, tag=f"acc{oct}_{sp}")
                        acc[oct, sp] = a_t

                for g in range(G):
                    t1, t2 = xt[img, g]
                    for oc4 in range(OC4):
                        iscl = g * OC4 + oc4
                        ratio_ap = scl[:, iscl : iscl + 1]
                        c_ap = scl[:, G * OC4 + iscl : G * OC4 + iscl + 1]
                        for sp in range(SP):
                            r0 = sp * ROWS
                            ps = pp.tile([128, NCOL], F32, tag="ps")
                            for s in range(3):
                                nc.tensor.matmul(
                                    ps[:],
                                    wslice(g, oc4, s),
                                    t1[:, :, r0 + s : r0 + s + ROWS, 0:W],
                                    start=(s == 0),
                                    stop=False,
                                    perf_mode=DR,
                                )
                            nc.tensor.matmul(
                                ps[:],
                                wslice(g, oc4, 3),
                                t2[:, :, r0 : r0 + ROWS, 2:PW],
                                start=False,
                                stop=False,
                                perf_mode=DR,
                            )
                            nc.tensor.matmul(
                                ps[:],
                                wslice(g, oc4, 4),
                                t1[:, :, r0 + 2 : r0 + 2 + ROWS, 2:PW],
                                start=False,
                                stop=True,
                                perf_mode=DR,
                            )
                            q8 = qp.tile([128, NCOL], I8, tag="q8")
                            nc.scalar.activation(
                                q8[:],
                                ps[:],
                                mybir.ActivationFunctionType.Copy,
                                bias=0.0,
                                scale=ratio_ap,
                            )
                            a = acc[oc4 % 2, sp]
                            if g == 0 and oc4 < 2:
                                nc.vector.tensor_scalar(
                                    a[:], q8[:], c_ap, None, mybir.AluOpType.mult
                                )
                            else:
                                nc.vector.scalar_tensor_tensor(
                                    a[:],
                                    q8[:],
                                    c_ap,
                                    a[:],
                                    mybir.AluOpType.mult,
                                    mybir.AluOpType.add,
                                )

                for oct in range(2):
                    for sp in range(SP):
                        a = acc[oct, sp]
                        eng = nc.sync if (oct * SP + sp) % 2 == 0 else nc.gpsimd
                        eng.dma_start(out_d[img, oct, sp], a[:])

    nc.compile()
    return nc


def _prepare(x, weight, w_scale, ps_scale_p, ps_scale_n):
    x = np.asarray(x, np.float32)
    weight = np.asarray(weight, np.float32)
    w_scale = np.asarray(w_scale, np.float32)
    ps_scale_p = np.asarray(ps_scale_p, np.float32)
    ps_scale_n = np.asarray(ps_scale_n, np.float32)

    # --- weight levels (exact f32 math matching the reference LSQ) ---
    wg = weight.reshape(OC, G, CG, K, K).transpose(1, 0, 2, 3, 4)  # [G,O,cg,k,k]
    s_w = w_scale.reshape(G, 1, 1, 1, 1)
    lvl_p = np.round(np.clip(np.maximum(wg, 0) / s_w, 0.0, float(QP_W))).astype(np.float32)
    lvl_n = np.round(np.clip(np.maximum(-wg, 0) / s_w, 0.0, float(QP_W))).astype(np.float32)
    LV = np.concatenate([lvl_p, lvl_n], axis=1)  # [G, 512, cg, 3, 3]

    # lhsT tiles [K=128, M=128] per (g, oc4, slot); both DoubleRow lanes get the
    # same integer-level weights (lane 0 contracts x_hi, lane 1 x_lo).
    wts = np.zeros((G, OC4, 5, 128, 128), np.float32)
    for g in range(G):
        for oc4 in range(OC4):
            t = LV[g, oc4 * 128 : (oc4 + 1) * 128]  # [128 och, cg, 3, 3]
            for s in range(3):  # taps (s,0)+(s,1)
                wts[g, oc4, s, :CG] = t[:, :, s, 0].T
                wts[g, oc4, s, CG:] = t[:, :, s, 1].T
            wts[g, oc4, 3, :CG] = t[:, :, 0, 2].T  # taps (0,2)+(1,2) via T2
            wts[g, oc4, 3, CG:] = t[:, :, 1, 2].T
            wts[g, oc4, 4, :CG] = t[:, :, 2, 2].T  # tap (2,2), upper half zero
    # -> [128 K, NSL, 2 lanes, 128 M]
    wflat = wts.transpose(3, 0, 1, 2, 4).reshape(128, NSL, 1, 128)
    wts_flat = np.ascontiguousarray(
        np.broadcast_to(wflat, (128, NSL, 2, 128))
    ).astype(NP_F8)

    # --- scales: ratio = s_w/s_ps ; c = +-s_ps ---
    scl = np.zeros((128, 2 * G * OC4), np.float32)
    for g in range(G):
        for oc4 in range(OC4):
            s_ps = ps_scale_p[g] if oc4 < 2 else ps_scale_n[g]
            sign = 1.0 if oc4 < 2 else -1.0
            scl[:, g * OC4 + oc4] = np.float32(w_scale[g]) / np.float32(s_ps)
            scl[:, G * OC4 + g * OC4 + oc4] = np.float32(sign) * np.float32(s_ps)

    # --- exact two-lane fp8 split of x: x ~ a/4 + b/128, lanes e4m3-exact ---
    # hi lane: integers/4 in [-4,4]; tail |x|>4.125 uses even ints (exact in
    # e4m3 up to 32) at step 1/2, with the lo lane also on an even grid there.
    a4 = np.round(x * 4)
    big = np.abs(a4) > 16
    a4 = np.where(big, 2 * np.round(x * 2), a4)
    a = a4.astype(np.float32) / 4
    r = x - a
    b128 = np.clip(np.round(r * 128), -16, 16)
    b128 = np.where(big, 2 * np.round(r * 64), b128)
    b = b128.astype(np.float32) / 128
    # padded, shifted lanes: [B, G, 128 part, 2 lane, 58, 58]
    # K-halves: [A | A+1col] for T1, [A | A+1row] for T2 (flat shifts by 1 / by PW;
    # the one flat-shift row-crossing read in pass 4's upper half has zero weights).
    T1 = np.zeros((B, G, 128, 2, FLAT), NP_F8)
    T2 = np.zeros((B, G, 128, 2, FLAT), NP_F8)
    for lane, xl in ((0, a), (1, b)):
        xp8 = np.zeros((B, IC, PH, PW), NP_F8)
        xp8[:, :, 1 : H + 1, 1 : W + 1] = xl.astype(NP_F8)
        Af = xp8.reshape(B, G, CG, FLAT)
        T1[:, :, :CG, lane] = Af
        T1[:, :, CG:, lane, : FLAT - 1] = Af[..., 1:]
        T2[:, :, :CG, lane] = Af
        T2[:, :, CG:, lane, : FLAT - PW] = Af[..., PW:]
    T1 = T1.reshape(B, G, 128, 2, PH, PW)
    T2 = T2.reshape(B, G, 128, 2, PH, PW)

    return T1, T2, wts_flat, scl


def kernel(x, weight, w_scale, ps_scale_p, ps_scale_n, _trace=False, _tmpdir=None):
    T1, T2, wts_flat, scl = _prepare(x, weight, w_scale, ps_scale_p, ps_scale_n)

    if "nc" not in _nc_cache:
        _nc_cache["nc"] = _build_nc()
    nc = _nc_cache["nc"]

    in_maps = []
    for c in range(N_CORES):
        sl = slice(c * BPC, (c + 1) * BPC)
        in_maps.append(
            {
                "xt1": np.ascontiguousarray(T1[sl]),
                "xt2": np.ascontiguousarray(T2[sl]),
                "wts": wts_flat,
                "scl": scl,
            }
        )

    kwargs = {}
    if _trace:
        kwargs.update(trace=True, tmpdir=_tmpdir, trace_cores=[0])
    res = run_bass_kernel_spmd(nc, in_maps, core_ids=list(range(N_CORES)), **kwargs)

    out = np.concatenate([r["out"] for r in res.results], axis=0)  # [16, 2, 7, 128, 448] fp16
    v = out.reshape(B, 2, SP, 128, ROWS, W)
    final = np.ascontiguousarray(v.transpose(0, 1, 3, 2, 4, 5)).reshape(B, OC, H, W).astype(np.float32)
    if _trace:
        kernel._last_results = res
    return final
